# revision 1
# baseline (speedup 1.0000x reference)
"""Trainium2 Bass kernel for EnergyPredTransformerGNN (3x TransformerConv + pool + MLP).

Sharding: nodes partitioned contiguously across 8 cores; edges sharded by dst
core; per-layer AllGather of node features; AllReduce of pooled graph features.

Self-contained: hardcodes full-problem sizes; host-side preprocessing only
reorders/pads index arrays and packs weights (no model math on host).
"""
import math
import sys

import numpy as np

sys.path.insert(0, "/opt/trn_rl_repo")

import concourse.bacc as bacc
import concourse.bass as bass
import concourse.tile as tile
from concourse import bass_utils, mybir
from concourse.bass import IndirectOffsetOnAxis
from concourse.masks import make_identity

P = 128
H, Dh, HD = 6, 32, 192
F32 = mybir.dt.float32
I32 = mybir.dt.int32
AF = mybir.ActivationFunctionType
OP = mybir.AluOpType
ISQ = 1.0 / math.sqrt(Dh)


class Cfg:
    def __init__(self, N=100000, E=400000, G=32, M=8, nt_e=None, bf16=True, abl=""):
        self.N, self.E, self.G, self.M = N, E, G, M
        self.bf16 = bf16
        self.abl = abl
        self.NS = N // M                      # real nodes per core
        assert N % M == 0
        self.NTN = (self.NS + P - 1) // P     # node tiles per core
        self.NL = self.NTN * P                # padded local nodes
        self.NPG = M * self.NL                # padded global nodes
        self.NT0 = self.NPG // P              # h0 tiles
        self.nt_e = nt_e                      # edge tiles per core (set by preprocess)

    def key(self):
        return (self.N, self.E, self.G, self.M, self.nt_e, self.bf16, self.abl)


# ---------------------------------------------------------------- host side
def preprocess(inputs, cfg):
    """Build per-core input maps. Index manipulation + weight packing only."""
    N, E, G, M, NS, NL = cfg.N, cfg.E, cfg.G, cfg.M, cfg.NS, cfg.NL
    x = np.asarray(inputs["x"], np.float32)
    ei = np.asarray(inputs["edge_index"]).astype(np.int64)
    ew = np.asarray(inputs["edge_weight"], np.float32).reshape(-1)
    batch = np.asarray(inputs["batch"]).astype(np.int64)
    ie = np.asarray(inputs["initial_energies"], np.float32)

    src, dst = ei[0], ei[1]
    core_of = dst // NS

    def gpad(n):
        c = n // NS
        return c * NL + (n - c * NS)

    SL = 64  # dst slots per edge tile
    per_core_tiles = []   # (tiles[nt,P] edge ids, d0[nt]) per core
    for c in range(M):
        sel = np.where(core_of == c)[0]
        d_loc = dst[sel] - c * NS
        order = np.argsort(d_loc, kind="stable")
        sel = sel[order]
        d_loc = d_loc[order]
        groups = np.split(sel, np.where(np.diff(d_loc))[0] + 1) if len(sel) else []
        tiles, d0s, cur = [], [], []
        cur_d0 = None
        for g in groups:
            gd = int(dst[g[0]] - c * NS)
            assert len(g) <= P, "in-degree > 128 unsupported"
            if cur and (len(cur) + len(g) > P or gd - cur_d0 >= SL):
                cur.extend([-1] * (P - len(cur)))
                tiles.append(cur)
                d0s.append(cur_d0)
                cur = []
            if not cur:
                cur_d0 = gd
            cur.extend(g.tolist())
        if cur:
            cur.extend([-1] * (P - len(cur)))
            tiles.append(cur)
            d0s.append(cur_d0)
        per_core_tiles.append((np.array(tiles, np.int64).reshape(-1, P),
                               np.array(d0s, np.int64)))
    nt_e = max(t[0].shape[0] for t in per_core_tiles)
    nt_e = (nt_e + 1) // 2 * 2
    if cfg.nt_e is None:
        cfg.nt_e = nt_e
    else:
        assert nt_e <= cfg.nt_e
    nt_e = cfg.nt_e

    x_pad = np.zeros((cfg.NPG, 4), np.float32)
    x_pad[gpad(np.arange(N))] = x

    w = {k: np.asarray(v, np.float32) for k, v in inputs.items()
         if k not in ("x", "edge_index", "edge_weight", "batch", "initial_energies")}

    def bc(row, parts=P):  # broadcast a [D] row to [parts, D]
        return np.repeat(np.asarray(row, np.float32).reshape(1, -1), parts, 0)

    com = {}
    com["Wp_s"] = w["Wp"]                                   # [4,192]
    com["bp_bc"] = bc(w["bp"])
    wkv = np.stack([
        np.concatenate([w["Wk"][i], w["Wv"][i]], 1).reshape(2, 96, 2 * HD)
        for i in range(3)])                                  # [3,2,96,384]
    if cfg.bf16:
        import ml_dtypes
        wkv = wkv.astype(ml_dtypes.bfloat16)
    com["Wkv"] = wkv
    com["Wqs"] = np.stack([
        np.concatenate([w["Wq"][i], w["Ws"][i]], 1).reshape(2, 96, 2 * HD)
        for i in range(3)])
    for nm, src_ in (("bq", w["bq"]), ("bs", w["bs"]), ("bk", w["bk"]),
                     ("We", w["We"][:, 0, :]), ("bv", w["bv"]),
                     ("lng", w["ln_g"]), ("lnb", w["ln_b"])):
        com[nm + "_bc"] = np.stack([bc(src_[i]) for i in range(3)])  # [3,128,192]
    com["iota_bc"] = bc(np.arange(G + 1, dtype=np.float32))          # [128,G+1]
    cnt = np.bincount(batch, minlength=G).astype(np.float32)
    com["invcnt"] = (1.0 / np.maximum(cnt, 1.0)).reshape(G, 1)
    com["ie_row"] = ie.reshape(1, G)
    com["fciW"] = w["fci_W"].reshape(1, HD)
    com["fcib"] = w["fci_b"].reshape(1, HD)
    com["fcig_bc"] = bc(w["fci_g"], G)
    com["fcilb_bc"] = bc(w["fci_lb"], G)
    com["fc1W"] = w["fc1_W"].reshape(3, P, HD)
    com["fc1b"] = w["fc1_b"].reshape(1, HD)
    com["fc1g_bc"] = bc(w["fc1_g"], G)
    com["fc1lb_bc"] = bc(w["fc1_lb"], G)
    com["fc2W"] = w["fc2_W"].reshape(2, 96, 96)
    com["fc2b"] = w["fc2_b"].reshape(1, 96)
    com["fc2g_bc"] = bc(w["fc2_g"], G)
    com["fc2lb_bc"] = bc(w["fc2_lb"], G)
    com["fc3W"] = w["fc3_W"].reshape(96, 1)
    com["fc3b"] = w["fc3_b"].reshape(1, 1)

    com["iota64_bc"] = bc(np.arange(SL, dtype=np.float32))   # [128,64]
    in_maps = []
    for c in range(M):
        tiles, d0s = per_core_tiles[c]
        nt_c = tiles.shape[0]
        eids = np.full((nt_e, P), -1, np.int64)
        eids[:nt_c] = tiles
        d0 = np.zeros((nt_e,), np.int64)
        d0[:nt_c] = d0s
        valid = eids >= 0
        e = np.where(valid, eids, 0)
        hgi = np.where(valid, gpad(src[e]), 0).astype(np.int32)[..., None]  # [nt,P,1]
        qidx = np.minimum(d0[:, None] + np.arange(SL)[None, :], NL - 1)
        qix = qidx.reshape(nt_e // 2, 2 * SL, 1).astype(np.int32)  # [nt/2,128,1]
        edf = np.zeros((nt_e, P, 2), np.float32)
        dm = np.where(valid, dst[e] - c * NS - d0[:, None], 127)
        edf[:, :, 0] = dm.astype(np.float32)
        edf[:, :, 1] = np.where(valid, ew[e], 0.0)
        # node -> aggtab2 row (tile*SL + slot); isolated/uncovered -> dummy block
        nodeslot = np.full((NL,), nt_e * SL, np.int64)
        for t in range(nt_c):
            ev = tiles[t][tiles[t] >= 0]
            dloc = dst[ev] - c * NS
            nodeslot[dloc] = t * SL + (dloc - d0s[t])
        ns2 = nodeslot.reshape(cfg.NTN, P).T.astype(np.int32)
        bf = np.full((NL, 1), float(G), np.float32)
        nloc = np.arange(NS)
        bf[nloc, 0] = batch[c * NS + nloc].astype(np.float32)
        m = dict(com)
        m["hgi"] = hgi
        m["qix"] = qix
        m["edf"] = edf
        m["batchf"] = bf
        m["x_loc"] = x_pad[c * NL:(c + 1) * NL]
        m["nodeslot"] = np.ascontiguousarray(ns2)  # [P, NTN]
        in_maps.append(m)
    return in_maps


# ---------------------------------------------------------------- device side
def build(cfg):
    NL, NPG, NTN, NT0, nt_e, G, M = (cfg.NL, cfg.NPG, cfg.NTN, cfg.NT0,
                                     cfg.nt_e, cfg.G, cfg.M)
    nc = bacc.Bacc("TRN2", target_bir_lowering=False, debug=False,
                   enable_asserts=False, num_devices=M)

    def inp(name, shape, dtype=F32):
        return nc.dram_tensor(name, list(shape), dtype, kind="ExternalInput").ap()

    BF16 = mybir.dt.bfloat16
    DT_H = BF16 if cfg.bf16 else F32
    SL = 64
    x_loc = inp("x_loc", (NL, 4))
    hgi = inp("hgi", (nt_e, P, 1), I32)
    qix = inp("qix", (nt_e // 2, 2 * SL, 1), I32)
    edf = inp("edf", (nt_e, P, 2))
    batchf = inp("batchf", (NL, 1))
    nodeslot = inp("nodeslot", (P, NTN), I32)
    iota64_bc = inp("iota64_bc", (P, SL))
    Wp_s = inp("Wp_s", (4, HD))
    bp_bc = inp("bp_bc", (P, HD))
    Wkv = inp("Wkv", (3, 2, 96, 2 * HD), DT_H)
    Wqs = inp("Wqs", (3, 2, 96, 2 * HD))
    LBC = {nm: inp(nm + "_bc", (3, P, HD))
           for nm in ("bq", "bs", "bk", "We", "bv", "lng", "lnb")}
    iota_bc = inp("iota_bc", (P, G + 1))
    invcnt = inp("invcnt", (G, 1))
    ie_row = inp("ie_row", (1, G))
    fciW = inp("fciW", (1, HD))
    fcib = inp("fcib", (1, HD))
    fcig_bc = inp("fcig_bc", (G, HD))
    fcilb_bc = inp("fcilb_bc", (G, HD))
    fc1W = inp("fc1W", (3, P, HD))
    fc1b = inp("fc1b", (1, HD))
    fc1g_bc = inp("fc1g_bc", (G, HD))
    fc1lb_bc = inp("fc1lb_bc", (G, HD))
    fc2W = inp("fc2W", (2, 96, 96))
    fc2b = inp("fc2b", (1, 96))
    fc2g_bc = inp("fc2g_bc", (G, 96))
    fc2lb_bc = inp("fc2lb_bc", (G, 96))
    fc3W = inp("fc3W", (96, 1))
    fc3b = inp("fc3b", (1, 1))

    out = nc.dram_tensor("out", [G, 1], F32, kind="ExternalOutput").ap()

    # internal DRAM
    hf = [nc.dram_tensor(f"hf{i}", [NPG, HD], DT_H, addr_space="Shared").ap()
          for i in range(3)]
    hshard = [nc.dram_tensor(f"hshard{i}", [NL, HD], DT_H).ap() for i in range(3)]
    hloc = [nc.dram_tensor(f"hloc{i}", [NL, HD], F32).ap() for i in range(3)]
    qtab = nc.dram_tensor("qtab", [NL, 204], DT_H).ap()
    sktab = nc.dram_tensor("sktab", [NL, HD], F32).ap()
    aggtab = nc.dram_tensor("aggtab", [nt_e * SL + P, 204], F32).ap()
    cc_in = nc.dram_tensor("cc_in", [G + 1, HD], F32).ap()
    cc_out = nc.dram_tensor("cc_out", [G + 1, HD], F32, addr_space="Shared").ap()

    from contextlib import ExitStack
    with tile.TileContext(nc) as tc, ExitStack() as es:
        cpool = es.enter_context(tc.tile_pool(name="consts", bufs=1))
        lpool = es.enter_context(tc.tile_pool(name="layerconsts", bufs=1))
        wk = es.enter_context(tc.tile_pool(name="work", bufs=4))
        wks = es.enter_context(tc.tile_pool(name="worksmall", bufs=8))
        ps_mm = es.enter_context(tc.tile_pool(name="psmm", bufs=2, space="PSUM"))
        ps_tr = es.enter_context(tc.tile_pool(name="pstr", bufs=3, space="PSUM"))
        ps_seg = es.enter_context(tc.tile_pool(name="psseg", bufs=2, space="PSUM"))
        ps_acc = es.enter_context(tc.tile_pool(name="psacc", bufs=1, space="PSUM"))
        hc = es.enter_context(tc.tile_pool(name="headc", bufs=1))

        ident = cpool.tile([P, P], F32)
        make_identity(nc, ident[:])
        ident_h = cpool.tile([P, P], DT_H)
        nc.vector.tensor_copy(out=ident_h[:], in_=ident[:])
        zero204 = cpool.tile([P, 204], F32)
        nc.gpsimd.memset(zero204[:], 0.0)
        eps_t = cpool.tile([P, 1], F32)
        nc.gpsimd.memset(eps_t[:], 1e-5)
        one_row = cpool.tile([1, P], F32)
        nc.gpsimd.memset(one_row[:], 1.0)
        Wp_sb = cpool.tile([4, HD], F32)
        nc.sync.dma_start(out=Wp_sb[:], in_=Wp_s[:, :])
        bp_sb = cpool.tile([P, HD], F32)
        nc.sync.dma_start(out=bp_sb[:], in_=bp_bc[:, :])
        iota_sb = cpool.tile([P, G + 1], F32)
        nc.sync.dma_start(out=iota_sb[:], in_=iota_bc[:, :])
        iota64_sb = cpool.tile([P, SL], F32)
        nc.sync.dma_start(out=iota64_sb[:], in_=iota64_bc[:, :])
        nodeslot_sb = cpool.tile([P, NTN], I32)
        nc.sync.dma_start(out=nodeslot_sb[:], in_=nodeslot[:, :])

        # ---------------- phase 0: h0 = x @ Wp + bp (local rows, then AllGather)
        for t in range(NTN):
            x_t = wks.tile([P, 4], F32, tag="x_t")
            nc.sync.dma_start(out=x_t[:], in_=x_loc[t * P:(t + 1) * P, :])
            xT_ps = ps_tr.tile([4, P], F32, tag="tr")
            nc.tensor.transpose(out=xT_ps[:], in_=x_t[:], identity=ident[:])
            xT_sb = wks.tile([4, P], F32, tag="xT_sb")
            nc.scalar.copy(out=xT_sb[:], in_=xT_ps[:])
            h0_ps = ps_mm.tile([P, 2 * HD], F32, tag="mm")
            nc.tensor.matmul(out=h0_ps[:, :HD], lhsT=xT_sb[:], rhs=Wp_sb[:],
                             start=True, stop=True)
            h0_sb = wk.tile([P, HD], F32, tag="h0_sb")
            nc.vector.tensor_add(out=h0_sb[:], in0=h0_ps[:, :HD], in1=bp_sb[:])
            nc.sync.dma_start(out=hloc[0][t * P:(t + 1) * P, :], in_=h0_sb[:])
            h0_bf = wk.tile([P, HD], DT_H, tag="h0_bf")
            nc.vector.tensor_copy(out=h0_bf[:], in_=h0_sb[:])
            nc.sync.dma_start(out=hshard[0][t * P:(t + 1) * P, :], in_=h0_bf[:])
        if M > 1 and "noag" not in cfg.abl:
            nc.gpsimd.collective_compute(
                "AllGather", OP.bypass, replica_groups=[list(range(M))],
                ins=[hshard[0][:, :]], outs=[hf[0][:, :]])
        elif M == 1:
            nc.sync.dma_start(out=hf[0][:, :], in_=hshard[0][:, :])

        pool_ps = ps_acc.tile([G + 1, HD], F32)

        # ---------------- 3 layers
        for L in range(3):
            hfL = hf[L]
            # layer consts
            Wkv_sb = [lpool.tile([96, 2 * HD], DT_H, tag=f"wkv{j}", name=f"wkv{j}") for j in range(2)]
            Wqs_sb = [lpool.tile([96, 2 * HD], F32, tag=f"wqs{j}", name=f"wqs{j}") for j in range(2)]
            for j in range(2):
                nc.sync.dma_start(out=Wkv_sb[j][:], in_=Wkv[L, j, :, :])
                nc.sync.dma_start(out=Wqs_sb[j][:], in_=Wqs[L, j, :, :])
            lsb = {}
            for nm in ("bq", "bs", "bk", "We", "bv", "lng", "lnb"):
                lsb[nm] = lpool.tile([P, HD], F32, tag=nm, name=nm)
                nc.sync.dma_start(out=lsb[nm][:], in_=LBC[nm][L, :, :])

            # zero only the dummy block (tile blocks are fully overwritten)
            nc.sync.dma_start(out=aggtab[nt_e * SL:nt_e * SL + P, :], in_=zero204[:])

            # ---- q / skip pass over local nodes (local rows of hfL sit at a
            # core-dependent offset; same program on all cores -> gather rows
            # via the per-core locidx column)
            for t in range(0 if "noq" in cfg.abl else NTN):
                h_t = wk.tile([P, HD], F32, tag="h_t")
                nc.sync.dma_start(out=h_t[:], in_=hloc[L][t * P:(t + 1) * P, :])
                hT_ps = [ps_tr.tile([96, P], F32, tag="tr", name=f"hT_ps{j2}") for j2 in range(2)]
                hT_sb = [wks.tile([96, P], F32, tag=f"hT{j2}", name=f"hT_sb{j2}") for j2 in range(2)]
                for j in range(2):
                    nc.tensor.transpose(out=hT_ps[j][:], in_=h_t[:, j * 96:(j + 1) * 96],
                                        identity=ident[:])
                    nc.scalar.copy(out=hT_sb[j][:], in_=hT_ps[j][:])
                qs_ps = ps_mm.tile([P, 2 * HD], F32, tag="mm")
                for j in range(2):
                    nc.tensor.matmul(out=qs_ps[:], lhsT=hT_sb[j][:], rhs=Wqs_sb[j][:],
                                     start=(j == 0), stop=(j == 1))
                qt_t = wk.tile([P, 204], DT_H, tag="qt_t")
                qf = wk.tile([P, HD], F32, tag="qf")
                nc.vector.tensor_add(out=qf[:], in0=qs_ps[:, :HD], in1=lsb["bq"][:])
                nc.vector.tensor_copy(out=qt_t[:, :HD], in_=qf[:])
                sk_t = wk.tile([P, HD], F32, tag="sk_t")
                nc.vector.tensor_add(out=sk_t[:], in0=qs_ps[:, HD:], in1=lsb["bs"][:])
                tmp = wk.tile([P, HD], F32, tag="qtmp")
                qbw = wks.tile([P, 12], F32, tag="qbw")
                nc.vector.tensor_mul(out=tmp[:], in0=qf[:], in1=lsb["bk"][:])
                nc.vector.tensor_reduce(out=qbw[:, 0:6],
                                        in_=tmp[:].rearrange("p (h d) -> p h d", d=Dh),
                                        axis=mybir.AxisListType.X, op=OP.add)
                nc.vector.tensor_mul(out=tmp[:], in0=qf[:], in1=lsb["We"][:])
                nc.vector.tensor_reduce(out=qbw[:, 6:12],
                                        in_=tmp[:].rearrange("p (h d) -> p h d", d=Dh),
                                        axis=mybir.AxisListType.X, op=OP.add)
                nc.vector.tensor_copy(out=qt_t[:, HD:HD + 12], in_=qbw[:])
                nc.sync.dma_start(out=qtab[t * P:(t + 1) * P, :], in_=qt_t[:])
                nc.sync.dma_start(out=sktab[t * P:(t + 1) * P, :], in_=sk_t[:])

            # ---- edge pass (qsl gathers paired: 2 tiles per indirect DMA)
            for g in range(0 if "noedge" in cfg.abl else nt_e // 2):
                qx2 = wks.tile([2 * SL, 1], I32, tag="qx2")
                nc.sync.dma_start(out=qx2[:], in_=qix[g, :, :])
                qsl2 = wk.tile([2 * SL, 204], DT_H, tag="qsl")
                nc.gpsimd.indirect_dma_start(
                    out=qsl2[:], out_offset=None, in_=qtab[:, :],
                    in_offset=IndirectOffsetOnAxis(ap=qx2[:, 0:1], axis=0))
                for j in range(2):
                    t = 2 * g + j
                    mt = wks.tile([P, 1], I32, tag="mt")
                    nc.sync.dma_start(out=mt[:], in_=hgi[t, :, :])
                    ed = wks.tile([P, 2], F32, tag="ed")
                    nc.sync.dma_start(out=ed[:], in_=edf[t, :, :])
                    hg = wk.tile([P, HD], DT_H, tag="hg")
                    nc.gpsimd.indirect_dma_start(
                        out=hg[:], out_offset=None, in_=hfL[:, :],
                        in_offset=IndirectOffsetOnAxis(ap=mt[:, 0:1], axis=0))
                    if "ehg" in cfg.abl:
                        continue
                    if j == 0:
                        qsl = qsl2[0:SL, :]
                    else:
                        qslB = wk.tile([SL, 204], DT_H, tag="qslB")
                        nc.sync.dma_start(out=qslB[:], in_=qsl2[SL:2 * SL, :])
                        qsl = qslB[:]
                    if "eqg" in cfg.abl:
                        continue
                    # S2[i,sl] = (dst_i - d0 == sl)
                    S2_sb = wk.tile([P, SL], DT_H, tag="S2_sb")
                    nc.vector.tensor_tensor(out=S2_sb[:],
                                            in0=ed[:, 0:1].to_broadcast([P, SL]),
                                            in1=iota64_sb[:], op=OP.is_equal)
                    S2T_ps = ps_tr.tile([SL, P], DT_H, tag="tr")
                    nc.tensor.transpose(out=S2T_ps[:], in_=S2_sb[:],
                                        identity=ident_h[:])
                    S2T_sb = wks.tile([SL, P], DT_H, tag="S2T_sb")
                    nc.scalar.copy(out=S2T_sb[:], in_=S2T_ps[:])
                    # expand q rows to edges: qe = S2 @ qsl
                    qe_ps = ps_seg.tile([P, 204], F32, tag="seg")
                    nc.tensor.matmul(out=qe_ps[:], lhsT=S2T_sb[:], rhs=qsl,
                                     start=True, stop=True)
                    qe = wk.tile([P, 204], DT_H, tag="qe_sb")
                    nc.scalar.copy(out=qe[:], in_=qe_ps[:])
                    hgT_ps = [ps_tr.tile([96, P], DT_H, tag="tr", name=f"hgT_ps{j2}") for j2 in range(2)]
                    hgT_sb = [wks.tile([96, P], DT_H, tag=f"hgT{j2}", name=f"hgT_sb{j2}") for j2 in range(2)]
                    for j2 in range(2):
                        nc.tensor.transpose(out=hgT_ps[j2][:],
                                            in_=hg[:, j2 * 96:(j2 + 1) * 96],
                                            identity=ident_h[:])
                        nc.scalar.copy(out=hgT_sb[j2][:], in_=hgT_ps[j2][:])
                    kv_ps = ps_mm.tile([P, 2 * HD], F32, tag="mm")
                    for j2 in range(2):
                        nc.tensor.matmul(out=kv_ps[:], lhsT=hgT_sb[j2][:],
                                         rhs=Wkv_sb[j2][:],
                                         start=(j2 == 0), stop=(j2 == 1))
                    prod = wk.tile([P, HD], F32, tag="prod")
                    nc.vector.tensor_mul(out=prod[:], in0=qe[:, :HD],
                                         in1=kv_ps[:, :HD])
                    lg2 = wks.tile([P, 6], F32, tag="lg2")
                    nc.vector.scalar_tensor_tensor(
                        out=lg2[:], in0=qe[:, HD + 6:HD + 12], scalar=ed[:, 1:2],
                        in1=qe[:, HD:HD + 6], op0=OP.mult, op1=OP.add)
                    lg = wks.tile([P, 6], F32, tag="lg")
                    nc.vector.tensor_reduce(out=lg[:],
                                            in_=prod[:].rearrange("p (h d) -> p h d", d=Dh),
                                            axis=mybir.AxisListType.X, op=OP.add)
                    nc.vector.tensor_add(out=lg[:], in0=lg[:], in1=lg2[:])
                    pu = wk.tile([P, 204], DT_H, tag="pu")
                    nc.scalar.activation(out=pu[:, 0:6], in_=lg[:], func=AF.Exp,
                                         scale=ISQ)
                    nc.vector.tensor_scalar_mul(out=pu[:, 6:12], in0=pu[:, 0:6],
                                                scalar1=ed[:, 1:2])
                    nc.vector.tensor_tensor(
                        out=pu[:, 12:].rearrange("p (h d) -> p h d", d=Dh),
                        in0=kv_ps[:, HD:].rearrange("p (h d) -> p h d", d=Dh),
                        in1=pu[:, 0:6].to_broadcast([P, 6, Dh]), op=OP.mult)
                    # per-slot segment sums: segd = S2.T @ pu  [SL, 204]
                    seg_ps = ps_seg.tile([SL, 204], F32, tag="seg")
                    nc.tensor.matmul(out=seg_ps[:], lhsT=S2_sb[:], rhs=pu[:],
                                     start=True, stop=True)
                    seg_sb = wk.tile([SL, 204], F32, tag="seg_sb")
                    nc.scalar.copy(out=seg_sb[:], in_=seg_ps[:])
                    nc.sync.dma_start(out=aggtab[t * SL:(t + 1) * SL, :],
                                      in_=seg_sb[:])

            # ---- node pass
            for t in range(NTN):
                ag = wk.tile([P, 204], F32, tag="ag")
                nc.gpsimd.indirect_dma_start(
                    out=ag[:], out_offset=None, in_=aggtab[:, :],
                    in_offset=IndirectOffsetOnAxis(ap=nodeslot_sb[:, t:t + 1], axis=0))
                sk_t = wk.tile([P, HD], F32, tag="sk2")
                nc.sync.dma_start(out=sk_t[:], in_=sktab[t * P:(t + 1) * P, :])
                h_t = wk.tile([P, HD], F32, tag="h_t2")
                nc.sync.dma_start(out=h_t[:], in_=hloc[L][t * P:(t + 1) * P, :])
                zz = wks.tile([P, 6], F32, tag="zz")
                nc.vector.tensor_scalar_add(out=zz[:], in0=ag[:, 0:6], scalar1=1e-30)
                rec = wks.tile([P, 6], F32, tag="rec")
                nc.vector.reciprocal(out=rec[:], in_=zz[:])
                w2r = wks.tile([P, 6], F32, tag="w2r")
                nc.vector.tensor_mul(out=w2r[:], in0=ag[:, 6:12], in1=rec[:])
                attn = wk.tile([P, HD], F32, tag="attn")
                nc.vector.tensor_tensor(
                    out=attn[:].rearrange("p (h d) -> p h d", d=Dh),
                    in0=ag[:, 12:].rearrange("p (h d) -> p h d", d=Dh),
                    in1=rec[:].to_broadcast([P, 6, Dh]), op=OP.mult)
                tmp = wk.tile([P, HD], F32, tag="ntmp")
                nc.vector.tensor_tensor(
                    out=tmp[:].rearrange("p (h d) -> p h d", d=Dh),
                    in0=lsb["We"][:].rearrange("p (h d) -> p h d", d=Dh),
                    in1=w2r[:].to_broadcast([P, 6, Dh]), op=OP.mult)
                nc.vector.tensor_add(out=attn[:], in0=attn[:], in1=tmp[:])
                hm = wks.tile([P, 1], F32, tag="hm")
                nc.vector.tensor_scalar(out=hm[:], in0=ag[:, 0:1], scalar1=0.0,
                                        scalar2=None, op0=OP.is_gt)
                nc.vector.tensor_scalar_mul(out=tmp[:], in0=lsb["bv"][:], scalar1=hm[:])
                nc.vector.tensor_add(out=attn[:], in0=attn[:], in1=tmp[:])
                nc.vector.tensor_add(out=attn[:], in0=attn[:], in1=sk_t[:])
                stats = wks.tile([P, 6], F32, tag="stats")
                nc.vector.bn_stats(out=stats[:], in_=attn[:])
                mv = wks.tile([P, 2], F32, tag="mv")
                nc.vector.bn_aggr(out=mv[:], in_=stats[:])
                nc.scalar.activation(out=mv[:, 1:2], in_=mv[:, 1:2], func=AF.Sqrt,
                                     bias=eps_t[:])
                nc.vector.reciprocal(out=mv[:, 1:2], in_=mv[:, 1:2])
                y = wk.tile([P, HD], F32, tag="y")
                nc.vector.tensor_scalar(out=y[:], in0=attn[:], scalar1=mv[:, 0:1],
                                        scalar2=mv[:, 1:2], op0=OP.subtract,
                                        op1=OP.mult)
                nc.vector.tensor_mul(out=y[:], in0=y[:], in1=lsb["lng"][:])
                nc.vector.tensor_add(out=y[:], in0=y[:], in1=lsb["lnb"][:])
                nc.scalar.activation(out=y[:], in_=y[:], func=AF.Relu)
                hn = wk.tile([P, HD], F32, tag="hn")
                nc.vector.tensor_add(out=hn[:], in0=h_t[:], in1=y[:])
                if L < 2:
                    nc.sync.dma_start(out=hloc[L + 1][t * P:(t + 1) * P, :], in_=hn[:])
                    hn_bf = wk.tile([P, HD], DT_H, tag="hn_bf")
                    nc.vector.tensor_copy(out=hn_bf[:], in_=hn[:])
                    nc.sync.dma_start(out=hshard[L + 1][t * P:(t + 1) * P, :],
                                      in_=hn_bf[:])
                else:
                    bf_t = wks.tile([P, 1], F32, tag="bf_t")
                    nc.sync.dma_start(out=bf_t[:], in_=batchf[t * P:(t + 1) * P, :])
                    B_sb = wks.tile([P, G + 1], F32, tag="B_sb")
                    nc.vector.tensor_tensor(out=B_sb[:],
                                            in0=bf_t[:, 0:1].to_broadcast([P, G + 1]),
                                            in1=iota_sb[:], op=OP.is_equal)
                    nc.tensor.matmul(out=pool_ps[:], lhsT=B_sb[:], rhs=hn[:],
                                     start=(t == 0), stop=(t == NTN - 1),
                                     skip_group_check=True)

            if L < 2:
                if M > 1 and "noag" not in cfg.abl:
                    nc.gpsimd.collective_compute(
                        "AllGather", OP.bypass,
                        replica_groups=[list(range(M))],
                        ins=[hshard[L + 1][:, :]], outs=[hf[L + 1][:, :]])
                elif M == 1:
                    nc.sync.dma_start(out=hf[L + 1][:, :], in_=hshard[L + 1][:, :])

        # ---------------- head
        pool_sb = hc.tile([G + 1, HD], F32, tag="pool_sb")
        nc.scalar.copy(out=pool_sb[:], in_=pool_ps[:])
        nc.sync.dma_start(out=cc_in[:, :], in_=pool_sb[:])
        if M > 1:
            nc.gpsimd.collective_compute(
                "AllReduce", OP.add, replica_groups=[list(range(M))],
                ins=[cc_in[:, :]], outs=[cc_out[:, :]])
            red_src = cc_out
        else:
            red_src = cc_in
        red_sb = hc.tile([G, HD], F32, tag="red_sb")
        nc.sync.dma_start(out=red_sb[:], in_=red_src[0:G, :])
        inv_sb = hc.tile([G, 1], F32, tag="inv_sb")
        nc.sync.dma_start(out=inv_sb[:], in_=invcnt[:, :])

        def head_const(ap_, shape, tag):
            t_ = hc.tile(list(shape), F32, tag=tag)
            nc.sync.dma_start(out=t_[:], in_=ap_[:, :] if len(shape) == 2 else ap_[:])
            return t_

        gf = hc.tile([G, HD], F32, tag="gf")
        nc.vector.tensor_scalar_mul(out=gf[:], in0=red_sb[:], scalar1=inv_sb[:])

        ie_sb = head_const(ie_row, (1, G), "ie_sb")
        fciW_sb = head_const(fciW, (1, HD), "fciW_sb")
        fcib_sb = head_const(fcib, (1, HD), "fcib_sb")
        if_ps = ps_mm.tile([G, HD], F32, tag="mm")
        nc.tensor.matmul(out=if_ps[:], lhsT=ie_sb[:], rhs=fciW_sb[:],
                         start=True, stop=False)
        nc.tensor.matmul(out=if_ps[:], lhsT=one_row[:, 0:G], rhs=fcib_sb[:],
                         start=False, stop=True)

        def ln_relu(src_ap, parts, width, g_sb, b_sb, tagp):
            st = hc.tile([parts, 6], F32, tag=tagp + "st")
            nc.vector.bn_stats(out=st[:], in_=src_ap)
            mv_ = hc.tile([parts, 2], F32, tag=tagp + "mv")
            nc.vector.bn_aggr(out=mv_[:], in_=st[:])
            nc.scalar.activation(out=mv_[:, 1:2], in_=mv_[:, 1:2], func=AF.Sqrt,
                                 bias=eps_t[0:parts, :])
            nc.vector.reciprocal(out=mv_[:, 1:2], in_=mv_[:, 1:2])
            o = hc.tile([parts, width], F32, tag=tagp + "o")
            nc.vector.tensor_scalar(out=o[:], in0=src_ap, scalar1=mv_[:, 0:1],
                                    scalar2=mv_[:, 1:2], op0=OP.subtract, op1=OP.mult)
            nc.vector.tensor_mul(out=o[:], in0=o[:], in1=g_sb[:])
            nc.vector.tensor_add(out=o[:], in0=o[:], in1=b_sb[:])
            nc.scalar.activation(out=o[:], in_=o[:], func=AF.Relu)
            return o

        fcig_sb = head_const(fcig_bc, (G, HD), "fcig_sb")
        fcilb_sb = head_const(fcilb_bc, (G, HD), "fcilb_sb")
        ifeat = ln_relu(if_ps[:], G, HD, fcig_sb, fcilb_sb, "ife")

        z_sb = hc.tile([G, 2 * HD], F32, tag="z_sb")
        nc.vector.tensor_copy(out=z_sb[:, :HD], in_=gf[:])
        nc.vector.tensor_copy(out=z_sb[:, HD:], in_=ifeat[:])

        fc1W_sb = [head_const(fc1W[k], (P, HD), f"fc1W{k}") for k in range(3)]
        fc1b_sb = head_const(fc1b, (1, HD), "fc1b_sb")
        z1_ps = ps_mm.tile([G, HD], F32, tag="mm")
        for k in range(3):
            zT_ps = ps_tr.tile([P, G], F32, tag="tr")
            nc.tensor.transpose(out=zT_ps[:], in_=z_sb[:, k * P:(k + 1) * P],
                                identity=ident[0:G, 0:G])
            zT_sb = hc.tile([P, G], F32, tag="zT_sb")
            nc.scalar.copy(out=zT_sb[:], in_=zT_ps[:])
            nc.tensor.matmul(out=z1_ps[:], lhsT=zT_sb[:], rhs=fc1W_sb[k][:],
                             start=(k == 0), stop=False)
        nc.tensor.matmul(out=z1_ps[:], lhsT=one_row[:, 0:G], rhs=fc1b_sb[:],
                         start=False, stop=True)
        fc1g_sb = head_const(fc1g_bc, (G, HD), "fc1g_sb")
        fc1lb_sb = head_const(fc1lb_bc, (G, HD), "fc1lb_sb")
        z1 = ln_relu(z1_ps[:], G, HD, fc1g_sb, fc1lb_sb, "z1")

        fc2W_sb = [head_const(fc2W[k], (96, 96), f"fc2W{k}") for k in range(2)]
        fc2b_sb = head_const(fc2b, (1, 96), "fc2b_sb")
        z2_ps = ps_mm.tile([G, 96], F32, tag="mm")
        for k in range(2):
            zT_ps = ps_tr.tile([96, G], F32, tag="tr")
            nc.tensor.transpose(out=zT_ps[:], in_=z1[:, k * 96:(k + 1) * 96],
                                identity=ident[0:G, 0:G])
            zT_sb = hc.tile([96, G], F32, tag="z2T_sb")
            nc.scalar.copy(out=zT_sb[:], in_=zT_ps[:])
            nc.tensor.matmul(out=z2_ps[:], lhsT=zT_sb[:], rhs=fc2W_sb[k][:],
                             start=(k == 0), stop=False)
        nc.tensor.matmul(out=z2_ps[:], lhsT=one_row[:, 0:G], rhs=fc2b_sb[:],
                         start=False, stop=True)
        fc2g_sb = head_const(fc2g_bc, (G, 96), "fc2g_sb")
        fc2lb_sb = head_const(fc2lb_bc, (G, 96), "fc2lb_sb")
        z2 = ln_relu(z2_ps[:], G, 96, fc2g_sb, fc2lb_sb, "z2")

        fc3W_sb = head_const(fc3W, (96, 1), "fc3W_sb")
        fc3b_sb = head_const(fc3b, (1, 1), "fc3b_sb")
        z3T_ps = ps_tr.tile([96, G], F32, tag="tr")
        nc.tensor.transpose(out=z3T_ps[:], in_=z2[:, :], identity=ident[0:G, 0:G])
        z3T_sb = hc.tile([96, G], F32, tag="z3T_sb")
        nc.scalar.copy(out=z3T_sb[:], in_=z3T_ps[:])
        o_ps = ps_mm.tile([G, 1], F32, tag="mm")
        nc.tensor.matmul(out=o_ps[:], lhsT=z3T_sb[:], rhs=fc3W_sb[:],
                         start=True, stop=False)
        nc.tensor.matmul(out=o_ps[:], lhsT=one_row[:, 0:G], rhs=fc3b_sb[:],
                         start=False, stop=True)
        o_sb = hc.tile([G, 1], F32, tag="o_sb")
        nc.scalar.copy(out=o_sb[:], in_=o_ps[:])
        nc.sync.dma_start(out=out[:, :], in_=o_sb[:])

    nc.compile()
    return nc


_CACHE = {}


def get_compiled(cfg):
    k = cfg.key()
    if k not in _CACHE:
        _CACHE[k] = build(cfg)
    return _CACHE[k]


def kernel(**inputs):
    cfg = Cfg()
    in_maps = preprocess(inputs, cfg)
    nc = get_compiled(cfg)
    res = bass_utils.run_bass_kernel_spmd(nc, in_maps, core_ids=list(range(cfg.M)))
    return np.asarray(res.results[0]["out"], np.float32)

